# revision 14
# baseline (speedup 1.0000x reference)
"""Trainium2 Bass kernel for nn_ConvBundle_48146583388363.

Math: out[x,y,b,i,j,o] = s[b, i+x-1, j+y-1] * wsum[x,y,o]
  where s = inputs.sum(channel) (zero-padded at borders) and
  wsum = W.sum(axis=2).

Sharding: data-parallel over batch B=16 across 8 cores (2 batches/core).
W and the small structural constants are replicated.

Per-core pipeline (output-bandwidth bound; ~43 MB of bf16 writes/core):
  1. x arrives host-transposed as [cin=64(p), 9216(f)] bf16 -> dense loads.
  2. ones[64,128]^T @ x_chunk matmuls channel-reduce AND broadcast s to all
     128 partitions in one PE op; ACT drains PSUM into a zero-padded bf16
     s vector [128, 97+9216+97] (s replicated per partition).
  3. Each tap shift is a pure AP offset into the padded s. A quarter-slab
     [128(cout), 2304(f)] is ONE dense DVE tensor_scalar_mul with the
     per-partition scalar wsumT[o] = W[tap].sum(cin)[o]; column-border
     masks are strided memsets of 24 columns.
  4. Slabs DMA out as [cout(p), f] bf16; host unshard transposes to
     [..., f, cout] and upcasts to f32 (rel-err of bf16 ~2e-3 << 2e-2).
"""

import numpy as np
import ml_dtypes

import concourse.bacc as bacc
import concourse.bass as bass
import concourse.mybir as mybir
from concourse import tile
from concourse.bass_utils import run_bass_kernel_spmd

F32 = mybir.dt.float32
BF16 = mybir.dt.bfloat16

NCORES = 8
B, H, W_, CIN = 16, 96, 96, 64
COUT = 128
BPC = B // NCORES          # batches per core = 2
SP = H * W_                # 9216 spatial positions per batch
NTAP = 9
TAPS = [(x - 1, y - 1) for x in range(3) for y in range(3)]  # tap n = 3x+y
PAD = 97                   # max |96*dx + dy|
CH = 512                   # s-broadcast matmul chunk = one PSUM bank
NCH = SP // CH             # 18
NQ = 4                     # output slab quarters
QF = SP // NQ              # 2304


def _build_nc():
    # Bacc (not raw Bass): its finalize() runs move_matmul_waits_to_ldweights
    # + generate_event_semaphores, which split multi-waits to satisfy the
    # 1-sync-wait-per-instruction hardware constraint.
    nc = bacc.Bacc(None, target_bir_lowering=False)
    # x viewed as [128, 4608]: partition 2c+h holds channel c, f-half h
    # (full 128-partition DMA spray; a [64, SP] layout runs at half rate).
    x = nc.dram_tensor("x", [BPC, 128, SP // 2], BF16, kind="ExternalInput")
    # w host-pretransposed to [cin, tap*cout]: one dense linear DMA. bf16 so
    # the wsumT matmuls are single-pass (f32 PE matmul = slow double-pass).
    w = nc.dram_tensor("w", [128, NTAP * COUT], BF16, kind="ExternalInput")
    # on[k, 128h:128h+128] = (k%2==h): lhsT masks that channel-sum the
    # even/odd partitions (= f-half h) of the x tile.
    on = nc.dram_tensor("on", [128, 256], BF16, kind="ExternalInput")
    # y stored (o, f) per (tap, batch): cout-major so each partition's 9216
    # bf16 values are one contiguous 18.4KB DRAM run; host transposes back.
    y = nc.dram_tensor("y", [NTAP, BPC, COUT, SP], BF16, kind="ExternalOutput")

    with tile.TileContext(nc) as tc:
        with (
            tc.tile_pool(name="const", bufs=1) as cpool,
            tc.tile_pool(name="xin", bufs=2) as xpool,
            tc.tile_pool(name="psum_w", bufs=1, space="PSUM") as pwpool,
            tc.tile_pool(name="psum_s", bufs=6, space="PSUM") as pspool,
            tc.tile_pool(name="out", bufs=12) as opool,
        ):
            # Batch 0 column-halves split across both HWDGE rings (it gates
            # the first slabs); consts on the sync ring; batch 1 on scalar.
            HC = SP // 4  # 2304 columns per half of the [128, 4608] tile
            xts = [
                xpool.tile([128, SP // 2], BF16, name=f"xt{b}", tag="xt")
                for b in range(BPC)
            ]
            # Consts first on the sync ring (tiny; gate the PE pipeline).
            # The ones column is memset on DVE -- a [128,1] DMA is 128 2-byte
            # descriptors, each paying full HBM latency (head-blocks the ring).
            on_sb = cpool.tile([128, 256], BF16, name="on_sb")
            nc.sync.dma_start(out=on_sb[:], in_=on[:, :])
            onf_sb = cpool.tile([128, 1], BF16, name="onf_sb")
            nc.vector.memset(onf_sb[:], 1.0)
            w_sb = cpool.tile([128, NTAP * COUT], BF16, name="w_sb")
            nc.sync.dma_start(out=w_sb[:], in_=w[:, :])
            nc.scalar.dma_start(out=xts[0][:, 0:HC], in_=x[0][:, 0:HC])
            nc.sync.dma_start(out=xts[0][:, HC:2 * HC], in_=x[0][:, HC:2 * HC])
            nc.scalar.dma_start(out=xts[1][:, 0:HC], in_=x[1][:, 0:HC])
            nc.scalar.dma_start(out=xts[1][:, HC:2 * HC], in_=x[1][:, HC:2 * HC])

            # Dummy matmul: pre-sync PE against the on_sb DMA lane so real
            # matmuls carry only their data-operand wait.
            junk = pwpool.tile([1, 1], F32, name="junk", tag="junk")
            nc.tensor.matmul(
                junk[:], lhsT=on_sb[:, 0:1], rhs=on_sb[:, 0:1],
                start=True, stop=True,
            )

            # wsumT[:, n] = colsum of W[n] with cout on partitions: 9 single-
            # pass matmuls into one PSUM tile, one ACT copy out (f32 scalar).
            wsumT = cpool.tile([128, NTAP], F32, name="wsumT")
            pwall = pwpool.tile([128, NTAP], F32, name="pwall", tag="pw")
            for n in range(NTAP):
                nc.tensor.matmul(
                    pwall[:, n:n + 1], lhsT=w_sb[:, n * COUT:(n + 1) * COUT],
                    rhs=onf_sb[:], start=True, stop=True,
                )
            # Drain on DVE: keeps ACT's in-order stream free for chunk copies.
            nc.vector.tensor_copy(wsumT[:], pwall[:])

            # s replicated across all 128 partitions, zero-padded both sides.
            # Chunk kk covers f in [512kk, 512kk+512): h = kk//9 picks the
            # even/odd lhsT mask, j = kk%9 the column chunk.
            svar = []
            for b in range(BPC):
                sv = cpool.tile([128, PAD + SP + PAD], BF16, name=f"sv{b}")
                nc.vector.memset(sv[:, 0:PAD], 0.0)
                nc.vector.memset(sv[:, PAD + SP:], 0.0)
                for kk in range(NCH):
                    h, j = kk // 9, kk % 9
                    ps = pspool.tile([128, CH], F32, name=f"ps{b}_{kk}", tag="ps")
                    nc.tensor.matmul(
                        ps[:], lhsT=on_sb[:, 128 * h:128 * (h + 1)],
                        rhs=xts[b][:, j * CH:(j + 1) * CH],
                        start=True, stop=True,
                    )
                    nc.scalar.copy(sv[:, PAD + kk * CH:PAD + (kk + 1) * CH], ps[:])
                svar.append(sv)

            # Center tap first within each quarter: earliest output DMA.
            order = sorted(range(NTAP), key=lambda n: TAPS[n] != (0, 0))
            for b in range(BPC):
                for q in range(NQ):
                    for n in order:
                        dx, dy = TAPS[n]
                        d = 96 * dx + dy
                        slab = opool.tile(
                            [128, QF], BF16, name=f"sl{n}_{b}_{q}", tag="slab"
                        )
                        nc.vector.tensor_scalar_mul(
                            slab[:],
                            svar[b][:, PAD + d + q * QF:PAD + d + (q + 1) * QF],
                            wsumT[:, n:n + 1],
                        )
                        if dy != 0:
                            j = 0 if dy == -1 else 95
                            nc.vector.memset(
                                slab[:].rearrange("p (i j) -> p i j", j=96)
                                [:, :, j:j + 1],
                                0.0,
                            )
                        nc.sync.dma_start(
                            out=y[n, b][:, q * QF:(q + 1) * QF], in_=slab[:]
                        )
    nc.finalize()
    return nc


_CACHE = {}


def _get_nc():
    if "nc" not in _CACHE:
        _CACHE["nc"] = _build_nc()
    return _CACHE["nc"]


def _run(x_full, w_full, **kwargs):
    nc = _get_nc()
    xr = x_full.reshape(NCORES, BPC, SP, CIN)
    ones = np.zeros((128, 256), ml_dtypes.bfloat16)
    ones[0::2, 0:128] = 1
    ones[1::2, 128:256] = 1
    wr = np.ascontiguousarray(
        w_full.reshape(NTAP, 128, COUT).transpose(1, 0, 2).reshape(128, NTAP * COUT)
    ).astype(ml_dtypes.bfloat16)
    in_maps = [
        {
            # host-side transpose: [sp, cin] -> [cin, sp], cast to bf16,
            # viewed [128, 4608] (partition 2c+h = channel c, f-half h)
            "x": xr[c].transpose(0, 2, 1).astype(ml_dtypes.bfloat16)
                 .reshape(BPC, 128, SP // 2),
            "w": wr,
            "on": ones,
        }
        for c in range(NCORES)
    ]
    return run_bass_kernel_spmd(nc, in_maps, core_ids=list(range(NCORES)), **kwargs)


def _unshard(results):
    """Per-core y is [9, BPC, 128(o), 9216(f)] bf16; transpose to
    [..., f, o] while gathering and upcast to f32."""
    out = np.empty((3, 3, B, H, W_, COUT), np.float32)
    ov = out.reshape(NTAP, B, SP, COUT)
    for c, r in enumerate(results):
        yc = np.asarray(r["y"]).astype(np.float32)
        ov[:, BPC * c:BPC * (c + 1)] = yc.transpose(0, 1, 3, 2)
    return out


def kernel(**inputs):
    x_full = np.ascontiguousarray(np.asarray(inputs["inputs"], dtype=np.float32))
    w_full = np.ascontiguousarray(np.asarray(inputs["W"], dtype=np.float32))
    res = _run(x_full, w_full)
    return _unshard(res.results)


# revision 16
# speedup vs baseline: 1.0472x; 1.0472x over previous
"""Trainium2 Bass kernel for nn_ConvBundle_48146583388363.

Math: out[x,y,b,i,j,o] = s[b, i+x-1, j+y-1] * wsum[x,y,o]
  where s = inputs.sum(channel) (zero-padded at borders) and
  wsum = W.sum(axis=2).

Sharding: data-parallel over batch B=16 across 8 cores (2 batches/core).
W and the small structural constants are replicated.

Per-core pipeline (output-bandwidth bound; ~43 MB of bf16 writes/core):
  1. x arrives host-transposed as [cin=64(p), 9216(f)] bf16 -> dense loads.
  2. ones[64,128]^T @ x_chunk matmuls channel-reduce AND broadcast s to all
     128 partitions in one PE op; ACT drains PSUM into a zero-padded bf16
     s vector [128, 97+9216+97] (s replicated per partition).
  3. Each tap shift is a pure AP offset into the padded s. A quarter-slab
     [128(cout), 2304(f)] is ONE dense DVE tensor_scalar_mul with the
     per-partition scalar wsumT[o] = W[tap].sum(cin)[o]; column-border
     masks are strided memsets of 24 columns.
  4. Slabs DMA out as [cout(p), f] bf16; host unshard transposes to
     [..., f, cout] and upcasts to f32 (rel-err of bf16 ~2e-3 << 2e-2).
"""

import numpy as np
import ml_dtypes

import concourse.bacc as bacc
import concourse.bass as bass
import concourse.mybir as mybir
from concourse import tile
from concourse.bass_utils import run_bass_kernel_spmd

F32 = mybir.dt.float32
BF16 = mybir.dt.bfloat16

NCORES = 8
B, H, W_, CIN = 16, 96, 96, 64
COUT = 128
BPC = B // NCORES          # batches per core = 2
SP = H * W_                # 9216 spatial positions per batch
NTAP = 9
TAPS = [(x - 1, y - 1) for x in range(3) for y in range(3)]  # tap n = 3x+y
PAD = 97                   # max |96*dx + dy|
CH = 512                   # s-broadcast matmul chunk = one PSUM bank
NCH = SP // CH             # 18
NQ = 4                     # output slab quarters
QF = SP // NQ              # 2304


def _build_nc():
    # Bacc (not raw Bass): its finalize() runs move_matmul_waits_to_ldweights
    # + generate_event_semaphores, which split multi-waits to satisfy the
    # 1-sync-wait-per-instruction hardware constraint.
    nc = bacc.Bacc(None, target_bir_lowering=False)
    # x viewed as [128, 4608]: partition 2c+h holds channel c, f-half h
    # (full 128-partition DMA spray; a [64, SP] layout runs at half rate).
    x = nc.dram_tensor("x", [BPC, 128, SP // 2], BF16, kind="ExternalInput")
    # w host-pretransposed to [cin, tap*cout]: one dense linear DMA. bf16 so
    # the wsumT matmuls are single-pass (f32 PE matmul = slow double-pass).
    w = nc.dram_tensor("w", [128, NTAP * COUT], BF16, kind="ExternalInput")
    # on[k, 128h:128h+128] = (k%2==h): lhsT masks that channel-sum the
    # even/odd partitions (= f-half h) of the x tile.
    on = nc.dram_tensor("on", [128, 256], BF16, kind="ExternalInput")
    # y stored (o, f) per (tap, batch): cout-major so each partition's 9216
    # bf16 values are one contiguous 18.4KB DRAM run; host transposes back.
    y = nc.dram_tensor("y", [NTAP, BPC, COUT, SP], BF16, kind="ExternalOutput")

    with tile.TileContext(nc) as tc:
        with (
            tc.tile_pool(name="const", bufs=1) as cpool,
            tc.tile_pool(name="xin", bufs=2) as xpool,
            tc.tile_pool(name="psum_w", bufs=1, space="PSUM") as pwpool,
            tc.tile_pool(name="psum_s", bufs=6, space="PSUM") as pspool,
            tc.tile_pool(name="out", bufs=12) as opool,
        ):
            # Batch 0 split across both HWDGE rings (it gates the first
            # slabs); consts on the sync ring; batch 1 on scalar. Asymmetric
            # split at col 2560 = chunk j=4 boundary: quarter-0 products need
            # exactly chunks j<=4, so piece A alone unblocks the first slabs.
            HC = 2560  # cols in piece A of the [128, 4608] tile
            xts = [
                xpool.tile([128, SP // 2], BF16, name=f"xt{b}", tag="xt")
                for b in range(BPC)
            ]
            # Consts first on the sync ring (tiny; gate the PE pipeline).
            # The ones column is memset on DVE -- a [128,1] DMA is 128 2-byte
            # descriptors, each paying full HBM latency (head-blocks the ring).
            on_sb = cpool.tile([128, 256], BF16, name="on_sb")
            nc.sync.dma_start(out=on_sb[:], in_=on[:, :])
            onf_sb = cpool.tile([128, 1], BF16, name="onf_sb")
            nc.vector.memset(onf_sb[:], 1.0)
            w_sb = cpool.tile([128, NTAP * COUT], BF16, name="w_sb")
            nc.sync.dma_start(out=w_sb[:], in_=w[:, :])
            NC2 = SP // 2  # 4608
            nc.scalar.dma_start(out=xts[0][:, 0:HC], in_=x[0][:, 0:HC])
            nc.sync.dma_start(out=xts[0][:, HC:NC2], in_=x[0][:, HC:NC2])
            nc.scalar.dma_start(out=xts[1][:, 0:HC], in_=x[1][:, 0:HC])
            nc.scalar.dma_start(out=xts[1][:, HC:NC2], in_=x[1][:, HC:NC2])

            # Dummy matmul: pre-sync PE against the on_sb DMA lane so real
            # matmuls carry only their data-operand wait.
            junk = pwpool.tile([1, 1], F32, name="junk", tag="junk")
            nc.tensor.matmul(
                junk[:], lhsT=on_sb[:, 0:1], rhs=on_sb[:, 0:1],
                start=True, stop=True,
            )

            # wsumT[:, n] = colsum of W[n] with cout on partitions: 9 single-
            # pass matmuls into one PSUM tile, one ACT copy out (f32 scalar).
            wsumT = cpool.tile([128, NTAP], F32, name="wsumT")
            pwall = pwpool.tile([128, NTAP], F32, name="pwall", tag="pw")
            for n in range(NTAP):
                nc.tensor.matmul(
                    pwall[:, n:n + 1], lhsT=w_sb[:, n * COUT:(n + 1) * COUT],
                    rhs=onf_sb[:], start=True, stop=True,
                )
            # Drain on DVE: keeps ACT's in-order stream free for chunk copies.
            nc.vector.tensor_copy(wsumT[:], pwall[:])

            # s replicated across all 128 partitions, zero-padded both sides.
            # Chunk kk covers f in [512kk, 512kk+512): h = kk//9 picks the
            # even/odd lhsT mask, j = kk%9 the column chunk.
            svar = []
            for b in range(BPC):
                sv = cpool.tile([128, PAD + SP + PAD], BF16, name=f"sv{b}")
                nc.vector.memset(sv[:, 0:PAD], 0.0)
                nc.vector.memset(sv[:, PAD + SP:], 0.0)
                for kk in range(NCH):
                    h, j = kk // 9, kk % 9
                    ps = pspool.tile([128, CH], F32, name=f"ps{b}_{kk}", tag="ps")
                    nc.tensor.matmul(
                        ps[:], lhsT=on_sb[:, 128 * h:128 * (h + 1)],
                        rhs=xts[b][:, j * CH:(j + 1) * CH],
                        start=True, stop=True,
                    )
                    nc.scalar.copy(sv[:, PAD + kk * CH:PAD + (kk + 1) * CH], ps[:])
                svar.append(sv)

            # Center tap first within each quarter: earliest output DMA.
            order = sorted(range(NTAP), key=lambda n: TAPS[n] != (0, 0))
            for b in range(BPC):
                for q in range(NQ):
                    for n in order:
                        dx, dy = TAPS[n]
                        d = 96 * dx + dy
                        slab = opool.tile(
                            [128, QF], BF16, name=f"sl{n}_{b}_{q}", tag="slab"
                        )
                        nc.vector.tensor_scalar_mul(
                            slab[:],
                            svar[b][:, PAD + d + q * QF:PAD + d + (q + 1) * QF],
                            wsumT[:, n:n + 1],
                        )
                        if dy != 0:
                            j = 0 if dy == -1 else 95
                            nc.vector.memset(
                                slab[:].rearrange("p (i j) -> p i j", j=96)
                                [:, :, j:j + 1],
                                0.0,
                            )
                        nc.sync.dma_start(
                            out=y[n, b][:, q * QF:(q + 1) * QF], in_=slab[:]
                        )
    nc.finalize()
    return nc


_CACHE = {}


def _get_nc():
    if "nc" not in _CACHE:
        _CACHE["nc"] = _build_nc()
    return _CACHE["nc"]


def _run(x_full, w_full, **kwargs):
    nc = _get_nc()
    xr = x_full.reshape(NCORES, BPC, SP, CIN)
    ones = np.zeros((128, 256), ml_dtypes.bfloat16)
    ones[0::2, 0:128] = 1
    ones[1::2, 128:256] = 1
    wr = np.ascontiguousarray(
        w_full.reshape(NTAP, 128, COUT).transpose(1, 0, 2).reshape(128, NTAP * COUT)
    ).astype(ml_dtypes.bfloat16)
    in_maps = [
        {
            # host-side transpose: [sp, cin] -> [cin, sp], cast to bf16,
            # viewed [128, 4608] (partition 2c+h = channel c, f-half h)
            "x": xr[c].transpose(0, 2, 1).astype(ml_dtypes.bfloat16)
                 .reshape(BPC, 128, SP // 2),
            "w": wr,
            "on": ones,
        }
        for c in range(NCORES)
    ]
    return run_bass_kernel_spmd(nc, in_maps, core_ids=list(range(NCORES)), **kwargs)


def _unshard(results):
    """Per-core y is [9, BPC, 128(o), 9216(f)] bf16; transpose to
    [..., f, o] while gathering and upcast to f32."""
    out = np.empty((3, 3, B, H, W_, COUT), np.float32)
    ov = out.reshape(NTAP, B, SP, COUT)
    for c, r in enumerate(results):
        yc = np.asarray(r["y"]).astype(np.float32)
        ov[:, BPC * c:BPC * (c + 1)] = yc.transpose(0, 1, 3, 2)
    return out


def kernel(**inputs):
    x_full = np.ascontiguousarray(np.asarray(inputs["inputs"], dtype=np.float32))
    w_full = np.ascontiguousarray(np.asarray(inputs["W"], dtype=np.float32))
    res = _run(x_full, w_full)
    return _unshard(res.results)


# revision 20
# speedup vs baseline: 1.2022x; 1.1480x over previous
"""Trainium2 Bass kernel for nn_ConvBundle_48146583388363.

Math: out[x,y,b,i,j,o] = s[b, i+x-1, j+y-1] * wsum[x,y,o]
  where s = inputs.sum(channel) (zero-padded at borders) and
  wsum = W.sum(axis=2).

Sharding: data-parallel over batch B=16 across 8 cores (2 batches/core).
W and the small structural constants are replicated.

Per-core pipeline (output-bandwidth bound; ~43 MB of bf16 writes/core):
  1. x arrives host-transposed as [cin=64(p), 9216(f)] bf16 -> dense loads.
  2. ones[64,128]^T @ x_chunk matmuls channel-reduce AND broadcast s to all
     128 partitions in one PE op; ACT drains PSUM into a zero-padded bf16
     s vector [128, 97+9216+97] (s replicated per partition).
  3. Each tap shift is a pure AP offset into the padded s. A quarter-slab
     [128(cout), 2304(f)] is ONE dense DVE tensor_scalar_mul with the
     per-partition scalar wsumT[o] = W[tap].sum(cin)[o]; column-border
     masks are strided memsets of 24 columns.
  4. Slabs DMA out as [cout(p), f] bf16; host unshard transposes to
     [..., f, cout] and upcasts to f32 (rel-err of bf16 ~2e-3 << 2e-2).
"""

import numpy as np
import ml_dtypes

import concourse.bacc as bacc
import concourse.bass as bass
import concourse.mybir as mybir
from concourse import tile
from concourse.bass_utils import run_bass_kernel_spmd

F32 = mybir.dt.float32
BF16 = mybir.dt.bfloat16

NCORES = 8
B, H, W_, CIN = 16, 96, 96, 64
COUT = 128
BPC = B // NCORES          # batches per core = 2
SP = H * W_                # 9216 spatial positions per batch
NTAP = 9
TAPS = [(x - 1, y - 1) for x in range(3) for y in range(3)]  # tap n = 3x+y
PAD = 97                   # max |96*dx + dy|
CH = 512                   # s-broadcast matmul chunk = one PSUM bank
NCH = SP // CH             # 18
NQ = 4                     # output slab quarters
QF = SP // NQ              # 2304


def _build_nc():
    # Bacc (not raw Bass): its finalize() runs move_matmul_waits_to_ldweights
    # + generate_event_semaphores, which split multi-waits to satisfy the
    # 1-sync-wait-per-instruction hardware constraint.
    nc = bacc.Bacc(None, target_bir_lowering=False)
    # x viewed as [128, 4608]: partition 2c+h holds channel c, f-half h
    # (full 128-partition DMA spray; a [64, SP] layout runs at half rate).
    x = nc.dram_tensor("x", [BPC, 128, SP // 2], BF16, kind="ExternalInput")
    # w host-pretransposed to [cin, tap*cout]: one dense linear DMA. bf16 so
    # the wsumT matmuls are single-pass (f32 PE matmul = slow double-pass).
    w = nc.dram_tensor("w", [128, NTAP * COUT], BF16, kind="ExternalInput")
    # on[k, 128h:128h+128] = (k%2==h): lhsT masks that channel-sum the
    # even/odd partitions (= f-half h) of the x tile.
    on = nc.dram_tensor("on", [128, 256], BF16, kind="ExternalInput")
    # y stored (o, f) per (tap, batch): cout-major so each partition's 9216
    # bf16 values are one contiguous 18.4KB DRAM run; host transposes back.
    y = nc.dram_tensor("y", [NTAP, BPC, COUT, SP], BF16, kind="ExternalOutput")

    with tile.TileContext(nc) as tc:
        with (
            tc.tile_pool(name="const", bufs=1) as cpool,
            tc.tile_pool(name="xin", bufs=2) as xpool,
            tc.tile_pool(name="psum_w", bufs=1, space="PSUM") as pwpool,
            tc.tile_pool(name="psum_s", bufs=6, space="PSUM") as pspool,
            tc.tile_pool(name="out", bufs=12) as opool,
        ):
            # Batch 0 split across both HWDGE rings (it gates the first
            # slabs); consts on the sync ring; batch 1 on scalar. Asymmetric
            # split at col 2560 = chunk j=4 boundary: quarter-0 products need
            # exactly chunks j<=4, so piece A alone unblocks the first slabs.
            HC = 2560  # cols in piece A of the [128, 4608] tile
            xts = [
                xpool.tile([128, SP // 2], BF16, name=f"xt{b}", tag="xt")
                for b in range(BPC)
            ]
            # Consts first on the sync ring (tiny; gate the PE pipeline).
            # The ones column is memset on DVE -- a [128,1] DMA is 128 2-byte
            # descriptors, each paying full HBM latency (head-blocks the ring).
            on_sb = cpool.tile([128, 256], BF16, name="on_sb")
            nc.sync.dma_start(out=on_sb[:], in_=on[:, :])
            onf_sb = cpool.tile([128, 1], BF16, name="onf_sb")
            nc.vector.memset(onf_sb[:], 1.0)
            w_sb = cpool.tile([128, NTAP * COUT], BF16, name="w_sb")
            nc.sync.dma_start(out=w_sb[:], in_=w[:, :])
            NC2 = SP // 2  # 4608
            nc.scalar.dma_start(out=xts[0][:, 0:1536], in_=x[0][:, 0:1536])
            nc.scalar.dma_start(out=xts[0][:, 1536:HC], in_=x[0][:, 1536:HC])
            nc.sync.dma_start(out=xts[0][:, HC:NC2], in_=x[0][:, HC:NC2])
            nc.scalar.dma_start(out=xts[1][:, 0:HC], in_=x[1][:, 0:HC])
            nc.scalar.dma_start(out=xts[1][:, HC:NC2], in_=x[1][:, HC:NC2])

            # Dummy matmul: pre-sync PE against the on_sb DMA lane so real
            # matmuls carry only their data-operand wait.
            junk = pwpool.tile([1, 1], F32, name="junk", tag="junk")
            nc.tensor.matmul(
                junk[:], lhsT=on_sb[:, 0:1], rhs=on_sb[:, 0:1],
                start=True, stop=True,
            )

            # wsumT[:, n] = colsum of W[n] with cout on partitions: 9 single-
            # pass matmuls into one PSUM tile, one DVE copy out (tensor_scalar
            # requires an f32 scalar operand).
            wsumT = cpool.tile([128, NTAP], F32, name="wsumT")
            pwall = pwpool.tile([128, NTAP], F32, name="pwall", tag="pw")
            for n in range(NTAP):
                nc.tensor.matmul(
                    pwall[:, n:n + 1], lhsT=w_sb[:, n * COUT:(n + 1) * COUT],
                    rhs=onf_sb[:], start=True, stop=True,
                )
            # Drain on DVE: keeps ACT's in-order stream free for chunk copies.
            nc.vector.tensor_copy(wsumT[:], pwall[:])

            # s replicated across all 128 partitions, zero-padded both sides.
            # Chunk kk covers f in [512kk, 512kk+512): h = kk//9 picks the
            # even/odd lhsT mask, j = kk%9 the column chunk.
            svar = []
            for b in range(BPC):
                sv = cpool.tile([128, PAD + SP + PAD], BF16, name=f"sv{b}")
                nc.vector.memset(sv[:, 0:PAD], 0.0)
                nc.vector.memset(sv[:, PAD + SP:], 0.0)
                for kk in range(NCH):
                    h, j = kk // 9, kk % 9
                    ps = pspool.tile([128, CH], F32, name=f"ps{b}_{kk}", tag="ps")
                    nc.tensor.matmul(
                        ps[:], lhsT=on_sb[:, 128 * h:128 * (h + 1)],
                        rhs=xts[b][:, j * CH:(j + 1) * CH],
                        start=True, stop=True,
                    )
                    dst = sv[:, PAD + kk * CH:PAD + (kk + 1) * CH]
                    # First-slab chain: alternate the earliest b0 drains onto
                    # DVE (idle pre-products) to halve sv-assembly latency.
                    if b == 0 and kk in (1, 3):
                        nc.vector.tensor_copy(dst, ps[:])
                    else:
                        nc.scalar.copy(dst, ps[:])
                svar.append(sv)

            # Center tap first within each quarter: earliest output DMA.
            order = sorted(range(NTAP), key=lambda n: TAPS[n] != (0, 0))
            for b in range(BPC):
                for q in range(NQ):
                    for n in order:
                        dx, dy = TAPS[n]
                        d = 96 * dx + dy
                        slab = opool.tile(
                            [128, QF], BF16, name=f"sl{n}_{b}_{q}", tag="slab"
                        )
                        nc.vector.tensor_scalar_mul(
                            slab[:],
                            svar[b][:, PAD + d + q * QF:PAD + d + (q + 1) * QF],
                            wsumT[:, n:n + 1],
                        )
                        if dy != 0:
                            j = 0 if dy == -1 else 95
                            nc.vector.memset(
                                slab[:].rearrange("p (i j) -> p i j", j=96)
                                [:, :, j:j + 1],
                                0.0,
                            )
                        nc.sync.dma_start(
                            out=y[n, b][:, q * QF:(q + 1) * QF], in_=slab[:]
                        )
    nc.finalize()
    return nc


_CACHE = {}


def _get_nc():
    if "nc" not in _CACHE:
        _CACHE["nc"] = _build_nc()
    return _CACHE["nc"]


def _run(x_full, w_full, **kwargs):
    nc = _get_nc()
    xr = x_full.reshape(NCORES, BPC, SP, CIN)
    ones = np.zeros((128, 256), ml_dtypes.bfloat16)
    ones[0::2, 0:128] = 1
    ones[1::2, 128:256] = 1
    wr = np.ascontiguousarray(
        w_full.reshape(NTAP, 128, COUT).transpose(1, 0, 2).reshape(128, NTAP * COUT)
    ).astype(ml_dtypes.bfloat16)
    in_maps = [
        {
            # host-side transpose: [sp, cin] -> [cin, sp], cast to bf16,
            # viewed [128, 4608] (partition 2c+h = channel c, f-half h)
            "x": xr[c].transpose(0, 2, 1).astype(ml_dtypes.bfloat16)
                 .reshape(BPC, 128, SP // 2),
            "w": wr,
            "on": ones,
        }
        for c in range(NCORES)
    ]
    return run_bass_kernel_spmd(nc, in_maps, core_ids=list(range(NCORES)), **kwargs)


def _unshard(results):
    """Per-core y is [9, BPC, 128(o), 9216(f)] bf16; transpose to
    [..., f, o] while gathering and upcast to f32."""
    out = np.empty((3, 3, B, H, W_, COUT), np.float32)
    ov = out.reshape(NTAP, B, SP, COUT)
    for c, r in enumerate(results):
        yc = np.asarray(r["y"]).astype(np.float32)
        ov[:, BPC * c:BPC * (c + 1)] = yc.transpose(0, 1, 3, 2)
    return out


def kernel(**inputs):
    x_full = np.ascontiguousarray(np.asarray(inputs["inputs"], dtype=np.float32))
    w_full = np.ascontiguousarray(np.asarray(inputs["W"], dtype=np.float32))
    res = _run(x_full, w_full)
    return _unshard(res.results)
